# revision 15
# baseline (speedup 1.0000x reference)
"""Causal attention (single head, d=1024) on 8 trn2 NeuronCores.

Problem: x[4,2048,1024], Wq/Wk/Wv[1024,1024] fp32;
out = softmax(mask(QK^T)/sqrt(1024)) @ V with mask j <= i+1.

Sharding: 2 cores per batch. Causal row work grows ~linearly with row
index, so the two cores split the 16 row-blocks of 128 as
{g : g%4 in {0,3}} vs {g : g%4 in {1,2}} (balanced). Each core receives
x[b] (transposed and natural orientation) with its own rows permuted to
the front so that every core runs the same SPMD program; causality is
enforced by a per-core additive mask tensor (data, not code).

Algebraic restructure (both projections eliminated from the device):
  scores: S = (x Wq)(x Wk)^T = x (Wq Wk^T) x^T.  M2 = Wq Wk^T is
    batch-independent and computed on the HOST; device computes
    U = x_own @ M2 then S = U @ x^T against the resident x^T.
  output: out = P (x Wv) = (P x) Wv.  Device computes PX = P @ x
    (against resident natural-orientation bf16 x), transposes PX, and
    projects once: out = PX @ Wv.  No K or V projection ever runs.

Precision: U/S/O matmuls run as single-pass float32r (PE truncates
reads to fp22 = e8m13, fp32 accumulate; 1 cycle/row at moving dim 512).
Host inputs are pre-rounded to fp22 RTN; U is rounded to fp22 RTN on
device via a Veltkamp split so the PE read is lossless. P and the PX
operand x are bf16 (output budget is lenient). Measured end-to-end
relative error ~7e-3 against the fp32 reference (gate 2e-2).

Attention is software-pipelined: softmax of row-block l overlaps score
matmuls of block l+1; fully-masked 128-col P blocks are skipped.
"""

import numpy as np
import ml_dtypes

import concourse.bass as bass
import concourse.mybir as mybir
import concourse.tile as tile
from concourse import bacc, masks
from concourse.bass_utils import run_bass_kernel_spmd

B, S, D, DA = 4, 2048, 1024, 1024
NCORES = 8
NBLK = S // 128  # 16 row blocks per batch
F32 = mybir.dt.float32
F32R = mybir.dt.float32r
BF16 = mybir.dt.bfloat16

ABLK = [g for g in range(NBLK) if g % 4 in (0, 3)]
BBLK = [g for g in range(NBLK) if g % 4 in (1, 2)]

NEG = -1e30


def _perm_rows(my):
    oth = [g for g in range(NBLK) if g not in my]
    idx = []
    for g in my + oth:
        idx.extend(range(g * 128, (g + 1) * 128))
    return np.array(idx, dtype=np.int64)


def _chunk_schedule():
    """Per local row-block l: which 512-col chunks of the permuted S row
    must be computed (union over the two roles, so the program is SPMD)."""
    sched = []
    for l in range(8):
        need = [False] * 4
        for my in (ABLK, BBLK):
            perm = _perm_rows(my)  # permuted col -> global row
            jmax = my[l] * 128 + 127 + 1  # max attended global col
            attended = perm <= jmax
            for ch in range(4):
                if attended[ch * 512 : (ch + 1) * 512].any():
                    need[ch] = True
        sched.append([ch for ch in range(4) if need[ch]])
    return sched


CHUNKS = _chunk_schedule()


def _pv_schedule():
    """Per local row-block l: which packed 128-col blocks of P (positions
    within the packed CHUNKS[l] layout) have any unmasked column for either
    role (union -> SPMD).  Blocks that are fully masked produce P == 0 and
    can be skipped in the P@x accumulation."""
    out = []
    for l in range(8):
        chunks = CHUNKS[l]
        needset = set()
        for my in (ABLK, BBLK):
            perm = _perm_rows(my)
            jmax = my[l] * 128 + 127 + 1
            attended = perm <= jmax
            for k, ch in enumerate(chunks):
                for q in range(4):
                    blk = ch * 4 + q
                    if attended[blk * 128 : (blk + 1) * 128].any():
                        needset.add(k * 4 + q)
        out.append(sorted(needset))
    return out


PVBLK = _pv_schedule()

_CACHE = {}


def _build():
    if "nc" in _CACHE:
        return _CACHE["nc"]

    nc = bacc.Bacc()
    # f32 inputs declared float32r (bit-identical) so non-casting DMA
    # queues can load them.
    xt_d = nc.dram_tensor("xt_perm", [D, S], F32R, kind="ExternalInput")
    m2_d = nc.dram_tensor("m2", [D, D], F32R, kind="ExternalInput")
    xb_d = nc.dram_tensor("xb_perm", [S, D], BF16, kind="ExternalInput")
    wv_d = nc.dram_tensor("wv", [D, DA], F32R, kind="ExternalInput")
    mask_d = nc.dram_tensor("maskb", [1024, S], BF16, kind="ExternalInput")
    out_d = nc.dram_tensor("out", [1024, DA], F32, kind="ExternalOutput")

    from contextlib import ExitStack

    with tile.TileContext(nc) as tc, ExitStack() as stack:
        cpool = stack.enter_context(tc.tile_pool(name="const", bufs=1))
        identb = cpool.tile([128, 128], BF16, tag="identb")
        masks.make_identity(nc, identb[:])

        # long-lived residents (live until the end of attention)
        xpool = stack.enter_context(tc.tile_pool(name="xtres", bufs=1))
        XT = [xpool.tile([128, S], F32R, name=f"xt{d}", tag=f"xt{d}") for d in range(8)]
        upool = stack.enter_context(tc.tile_pool(name="utres", bufs=1))
        UT = [upool.tile([128, 1024], F32R, name=f"ut{a}", tag=f"ut{a}") for a in range(8)]
        bpool = stack.enter_context(tc.tile_pool(name="xbres", bufs=1))
        XB = [bpool.tile([128, D], BF16, name=f"xb{j}", tag=f"xb{j}") for j in range(16)]
        wpool = stack.enter_context(tc.tile_pool(name="wvres", bufs=1))
        WV = [wpool.tile([128, DA], F32R, name=f"wv{d}", tag=f"wv{d}") for d in range(8)]

        # x^T loaded once, in 512-col chunks so the U phase starts early
        for jc in range(4):
            for d in range(8):
                nc.sync.dma_start(
                    XT[d][:, jc * 512 : (jc + 1) * 512],
                    xt_d[d * 128 : (d + 1) * 128, jc * 512 : (jc + 1) * 512],
                )
        # natural-orientation bf16 x rows (for P@x) and Wv (for PX@Wv):
        # needed only once attention starts, loaded behind the U phase.
        for j in range(16):
            nc.scalar.dma_start(XB[j][:], xb_d[j * 128 : (j + 1) * 128, :])
        for d in range(8):
            nc.scalar.dma_start(WV[d][:], wv_d[d * 128 : (d + 1) * 128, :])

        # ---- U^T = M2^T x_own^T (own rows = first two chunks) -------------
        with (
            tc.tile_pool(name="m2w", bufs=1) as pm,
            tc.tile_pool(name="vtmp", bufs=2) as ptmp,
            tc.tile_pool(name="psproj", bufs=4, space="PSUM") as pps,
        ):
            m2 = [pm.tile([128, D], F32R, name=f"m2{d}", tag=f"m2{d}") for d in range(8)]
            # column-sliced loads in a-major order: the first U psum group
            # (a=0) needs only the first 128 cols of every m2 tile.
            for a in range(8):
                for d in range(8):
                    nc.gpsimd.dma_start(
                        m2[d][:, a * 128 : (a + 1) * 128],
                        m2_d[d * 128 : (d + 1) * 128, a * 128 : (a + 1) * 128],
                    )

            def round13(dst, ps):
                # Veltkamp split: round PSUM fp32 to 14-bit significand
                # (e8m13) round-to-nearest, so the PE's f32r read of dst is
                # lossless.
                c = ptmp.tile([128, 512], F32, tag="vc")
                dd = ptmp.tile([128, 512], F32, tag="vd")
                nc.vector.tensor_scalar_mul(c[:], ps[:], 1025.0)
                nc.vector.tensor_sub(dd[:], c[:], ps[:])
                nc.vector.tensor_sub(dst, c[:], dd[:])

            for jc in range(2):
                csl = slice(jc * 512, (jc + 1) * 512)
                for a in range(8):
                    ps = pps.tile([128, 512], F32, tag="psp")
                    for d in range(8):
                        nc.tensor.matmul(
                            ps[:],
                            m2[d][:, a * 128 : (a + 1) * 128],
                            XT[d][:, csl],
                            start=(d == 0),
                            stop=(d == 7),
                        )
                    round13(UT[a][:, csl], ps)

        # ---- Attention per local row-block, software-pipelined -----------
        with (
            tc.tile_pool(name="psS", bufs=2, space="PSUM") as psS,
            tc.tile_pool(name="psPX", bufs=1, space="PSUM") as psPX,
            tc.tile_pool(name="psO", bufs=1, space="PSUM") as psO,
            tc.tile_pool(name="psT", bufs=1, space="PSUM") as psT,
            tc.tile_pool(name="psTx", bufs=1, space="PSUM") as psTx,
            tc.tile_pool(name="attn", bufs=2) as pa,
            tc.tile_pool(name="attn1", bufs=2) as pa1,
            tc.tile_pool(name="pxsb", bufs=2) as ppx,
        ):
            # stage state carried from score/softmax stage to PX stage
            state = {}

            def emit_scores(l):
                chunks = CHUNKS[l]
                W = len(chunks) * 512
                lsl = slice(l * 128, (l + 1) * 128)
                S_sb = pa.tile([128, 2048], F32, tag="S")
                for k, ch in enumerate(chunks):
                    ps = psS.tile([128, 512], F32, tag="ps")
                    csl = slice(ch * 512, (ch + 1) * 512)
                    for ac in range(8):
                        nc.tensor.matmul(
                            ps[:],
                            UT[ac][:, lsl],
                            XT[ac][:, csl],
                            start=(ac == 0),
                            stop=(ac == 7),
                        )
                    mk = pa1.tile([128, 512], BF16, tag="mk")
                    nc.gpsimd.dma_start(mk[:], mask_d[lsl, csl])
                    nc.vector.tensor_add(S_sb[:, k * 512 : (k + 1) * 512], ps[:], mk[:])

                mx = pa1.tile([128, 1], F32, tag="mx")
                nc.vector.reduce_max(mx[:], S_sb[:, 0:W], axis=mybir.AxisListType.X)
                negb = pa1.tile([128, 1], F32, tag="negb")
                nc.vector.tensor_scalar_mul(negb[:], mx[:], -1.0 / 32.0)
                P_sb = pa.tile([128, 2048], BF16, tag="P")
                rs = pa1.tile([128, 1], F32, tag="rs")
                nc.scalar.activation(
                    P_sb[:, 0:W],
                    S_sb[:, 0:W],
                    mybir.ActivationFunctionType.Exp,
                    bias=negb[:],
                    scale=1.0 / 32.0,
                    accum_out=rs[:],
                )
                state[l] = (P_sb, rs)

            def emit_px(l):
                chunks = CHUNKS[l]
                lsl = slice(l * 128, (l + 1) * 128)
                P_sb, rs = state.pop(l)

                # PX = P @ x  (accumulate over unmasked 128-col blocks)
                pxacc = [psPX.tile([128, 512], F32, name=f"px{h}", tag=f"px{h}") for h in range(2)]
                blocks = PVBLK[l]
                for i, q in enumerate(blocks):
                    vj = chunks[q // 4] * 4 + (q % 4)
                    pst = psT.tile([128, 128], BF16, tag="pst")
                    nc.tensor.transpose(
                        pst[:], P_sb[:, q * 128 : (q + 1) * 128], identb[:]
                    )
                    pt = pa1.tile([128, 128], BF16, tag="pt")
                    nc.vector.tensor_copy(pt[:], pst[:])
                    for half in range(2):
                        nc.tensor.matmul(
                            pxacc[half][:],
                            pt[:],
                            XB[vj][:, half * 512 : (half + 1) * 512],
                            start=(i == 0),
                            stop=(i == len(blocks) - 1),
                        )

                # PX -> SBUF, transpose to PX^T, project: out = PX @ Wv
                px = ppx.tile([128, 1024], F32, tag="px")
                for half in range(2):
                    nc.vector.tensor_copy(
                        px[:, half * 512 : (half + 1) * 512], pxacc[half][:]
                    )
                # transpose PX 128-col blocks and project each against Wv;
                # interleaving O matmuls with transposes keeps the single-
                # buffer transpose bank from stalling the PE.
                oacc = [psO.tile([128, 512], F32, name=f"oacc{h}", tag=f"oacc{h}") for h in range(2)]
                for dc in range(8):
                    pstx = psTx.tile([128, 128], F32, tag="pstx")
                    nc.tensor.transpose(
                        pstx[:], px[:, dc * 128 : (dc + 1) * 128], identf[:]
                    )
                    t = pa1.tile([128, 128], F32R, tag="pxt")
                    nc.vector.tensor_copy(t[:], pstx[:])
                    for half in range(2):
                        nc.tensor.matmul(
                            oacc[half][:],
                            t[:],
                            WV[dc][:, half * 512 : (half + 1) * 512],
                            start=(dc == 0),
                            stop=(dc == 7),
                        )

                rec = pa1.tile([128, 1], F32, tag="rec")
                nc.vector.reciprocal(rec[:], rs[:])
                for half in range(2):
                    o_sb = pa1.tile([128, 512], F32, tag="o")
                    nc.vector.tensor_scalar_mul(o_sb[:], oacc[half][:], rec[:])
                    nc.sync.dma_start(
                        out_d[lsl, half * 512 : (half + 1) * 512],
                        o_sb[:],
                    )

            # f32 identity for the PX^T transposes
            identf = pa.tile([128, 128], F32, tag="identf")
            masks.make_identity(nc, identf[:])

            for l in range(9):
                if l < 8:
                    emit_scores(l)
                if l >= 1:
                    emit_px(l - 1)

    nc.compile()
    _CACHE["nc"] = nc
    return nc


def _rtn22(a):
    """Round fp32 to fp22 (e8m13) with round-to-nearest on host.  The PE
    reads f32r operands truncated to fp22; pre-rounding makes that read
    lossless and replaces truncation bias with unbiased RTN error."""
    u = np.ascontiguousarray(a, dtype=np.float32).view(np.uint32)
    u = (u + np.uint32(0x200)) & np.uint32(0xFFFFFC00)
    return u.view(np.float32)


def _core_inputs(x, Wq, Wk, Wv, c):
    b = c // 2
    my = ABLK if c % 2 == 0 else BBLK
    perm = _perm_rows(my)
    gi = np.concatenate([np.arange(g * 128, (g + 1) * 128) for g in my])
    mask = np.where(perm[None, :] <= gi[:, None] + 1, 0.0, NEG).astype(
        ml_dtypes.bfloat16
    )
    key = ("m2", id(Wq), id(Wk))
    if _CACHE.get("m2key") != key:
        _CACHE["m2"] = _rtn22(
            (Wq.astype(np.float64) @ Wk.T.astype(np.float64)).astype(np.float32)
        )
        _CACHE["m2key"] = key
    xp = x[b][perm]
    return {
        "xt_perm": _rtn22(np.ascontiguousarray(xp.T)),
        "m2": _CACHE["m2"],
        "xb_perm": np.ascontiguousarray(xp).astype(ml_dtypes.bfloat16),
        "wv": _rtn22(Wv),
        "maskb": mask,
    }, (b, my)


def kernel(x, Wq, Wk, Wv):
    x = np.ascontiguousarray(np.asarray(x, dtype=np.float32))
    Wq = np.ascontiguousarray(np.asarray(Wq, dtype=np.float32))
    Wk = np.ascontiguousarray(np.asarray(Wk, dtype=np.float32))
    Wv = np.ascontiguousarray(np.asarray(Wv, dtype=np.float32))

    nc = _build()

    in_maps = []
    metas = []
    for c in range(NCORES):
        m, meta = _core_inputs(x, Wq, Wk, Wv, c)
        in_maps.append(m)
        metas.append(meta)

    res = run_bass_kernel_spmd(nc, in_maps, list(range(NCORES)))

    out = np.empty((B, S, DA), dtype=np.float32)
    for c in range(NCORES):
        b, my = metas[c]
        o = res.results[c]["out"]
        for l, g in enumerate(my):
            out[b, g * 128 : (g + 1) * 128] = o[l * 128 : (l + 1) * 128]
    return out


# revision 18
# speedup vs baseline: 1.1000x; 1.1000x over previous
"""Causal attention (single head, d=1024) on 8 trn2 NeuronCores.

Problem: x[4,2048,1024], Wq/Wk/Wv[1024,1024] fp32;
out = softmax(mask(QK^T)/sqrt(1024)) @ V with mask j <= i+1.

Sharding: 2 cores per batch. Causal row work grows ~linearly with row
index, so the two cores split the 16 row-blocks of 128 as
{g : g%4 in {0,3}} vs {g : g%4 in {1,2}} (balanced). Each core receives
x[b] (transposed and natural orientation) with its own rows permuted to
the front so that every core runs the same SPMD program; causality is
enforced by a per-core additive mask tensor (data, not code).

Algebraic restructure (both projections eliminated from the device):
  scores: S = (x Wq)(x Wk)^T = x (Wq Wk^T) x^T.  M2 = Wq Wk^T is
    batch-independent and computed on the HOST; device computes
    U = x_own @ M2 then S = U @ x^T against the resident x^T.
  output: out = P (x Wv) = (P x) Wv.  Device computes PX = P @ x
    (against resident natural-orientation bf16 x), transposes PX, and
    projects once: out = PX @ Wv.  No K or V projection ever runs.

Precision: U/S/O matmuls run as single-pass float32r (PE truncates
reads to fp22 = e8m13, fp32 accumulate; 1 cycle/row at moving dim 512).
Host inputs are pre-rounded to fp22 RTN; U is rounded to fp22 RTN on
device via a Veltkamp split so the PE read is lossless. P and the PX
operand x are bf16 (output budget is lenient). Measured end-to-end
relative error ~7e-3 against the fp32 reference (gate 2e-2).

Attention is software-pipelined: softmax of row-block l overlaps score
matmuls of block l+1; fully-masked 128-col P blocks are skipped.
"""

import numpy as np
import ml_dtypes

import concourse.bass as bass
import concourse.mybir as mybir
import concourse.tile as tile
from concourse import bacc, masks
from concourse.bass_utils import run_bass_kernel_spmd

B, S, D, DA = 4, 2048, 1024, 1024
NCORES = 8
NBLK = S // 128  # 16 row blocks per batch
F32 = mybir.dt.float32
F32R = mybir.dt.float32r
BF16 = mybir.dt.bfloat16

ABLK = [g for g in range(NBLK) if g % 4 in (0, 3)]
BBLK = [g for g in range(NBLK) if g % 4 in (1, 2)]

NEG = -1e30


def _perm_rows(my):
    oth = [g for g in range(NBLK) if g not in my]
    idx = []
    for g in my + oth:
        idx.extend(range(g * 128, (g + 1) * 128))
    return np.array(idx, dtype=np.int64)


def _chunk_schedule():
    """Per local row-block l: which 512-col chunks of the permuted S row
    must be computed (union over the two roles, so the program is SPMD)."""
    sched = []
    for l in range(8):
        need = [False] * 4
        for my in (ABLK, BBLK):
            perm = _perm_rows(my)  # permuted col -> global row
            jmax = my[l] * 128 + 127 + 1  # max attended global col
            attended = perm <= jmax
            for ch in range(4):
                if attended[ch * 512 : (ch + 1) * 512].any():
                    need[ch] = True
        sched.append([ch for ch in range(4) if need[ch]])
    return sched


CHUNKS = _chunk_schedule()


def _pv_schedule():
    """Per local row-block l: which packed 128-col blocks of P (positions
    within the packed CHUNKS[l] layout) have any unmasked column for either
    role (union -> SPMD).  Blocks that are fully masked produce P == 0 and
    can be skipped in the P@x accumulation."""
    out = []
    for l in range(8):
        chunks = CHUNKS[l]
        needset = set()
        for my in (ABLK, BBLK):
            perm = _perm_rows(my)
            jmax = my[l] * 128 + 127 + 1
            attended = perm <= jmax
            for k, ch in enumerate(chunks):
                for q in range(4):
                    blk = ch * 4 + q
                    if attended[blk * 128 : (blk + 1) * 128].any():
                        needset.add(k * 4 + q)
        out.append(sorted(needset))
    return out


PVBLK = _pv_schedule()

_CACHE = {}


def _build():
    if "nc" in _CACHE:
        return _CACHE["nc"]

    nc = bacc.Bacc()
    # f32 inputs declared float32r (bit-identical) so non-casting DMA
    # queues can load them.
    xt_d = nc.dram_tensor("xt_perm", [D, S], F32R, kind="ExternalInput")
    m2_d = nc.dram_tensor("m2", [D, D], F32R, kind="ExternalInput")
    xb_d = nc.dram_tensor("xb_perm", [S, D], BF16, kind="ExternalInput")
    wv_d = nc.dram_tensor("wv", [D, DA], F32R, kind="ExternalInput")
    mask_d = nc.dram_tensor("maskb", [1024, S], BF16, kind="ExternalInput")
    out_d = nc.dram_tensor("out", [1024, DA], F32, kind="ExternalOutput")

    from contextlib import ExitStack

    with tile.TileContext(nc) as tc, ExitStack() as stack:
        cpool = stack.enter_context(tc.tile_pool(name="const", bufs=1))
        identb = cpool.tile([128, 128], BF16, tag="identb")
        masks.make_identity(nc, identb[:])

        # long-lived residents (live until the end of attention)
        xpool = stack.enter_context(tc.tile_pool(name="xtres", bufs=1))
        XT = [xpool.tile([128, S], F32R, name=f"xt{d}", tag=f"xt{d}") for d in range(8)]
        upool = stack.enter_context(tc.tile_pool(name="utres", bufs=1))
        UT = [upool.tile([128, 1024], F32R, name=f"ut{a}", tag=f"ut{a}") for a in range(8)]
        bpool = stack.enter_context(tc.tile_pool(name="xbres", bufs=1))
        XB = [bpool.tile([128, D], BF16, name=f"xb{j}", tag=f"xb{j}") for j in range(16)]
        wpool = stack.enter_context(tc.tile_pool(name="wvres", bufs=1))
        WV = [wpool.tile([128, DA], F32R, name=f"wv{d}", tag=f"wv{d}") for d in range(8)]

        # x^T loaded once, in 512-col chunks so the U phase starts early
        for jc in range(4):
            for d in range(8):
                nc.sync.dma_start(
                    XT[d][:, jc * 512 : (jc + 1) * 512],
                    xt_d[d * 128 : (d + 1) * 128, jc * 512 : (jc + 1) * 512],
                )
        # ---- U^T = M2^T x_own^T (own rows = first two chunks) -------------
        with (
            tc.tile_pool(name="m2w", bufs=1) as pm,
            tc.tile_pool(name="vtmp", bufs=2) as ptmp,
            tc.tile_pool(name="psproj", bufs=4, space="PSUM") as pps,
        ):
            m2 = [pm.tile([128, D], F32R, name=f"m2{d}", tag=f"m2{d}") for d in range(8)]
            # column-sliced loads in a-major order: the first U psum group
            # (a=0) needs only the first 128 cols of every m2 tile.
            for a in range(8):
                for d in range(8):
                    nc.gpsimd.dma_start(
                        m2[d][:, a * 128 : (a + 1) * 128],
                        m2_d[d * 128 : (d + 1) * 128, a * 128 : (a + 1) * 128],
                    )

            def round13(dst, ps):
                # Veltkamp split: round PSUM fp32 to 14-bit significand
                # (e8m13) round-to-nearest, so the PE's f32r read of dst is
                # lossless.
                c = ptmp.tile([128, 512], F32, tag="vc")
                dd = ptmp.tile([128, 512], F32, tag="vd")
                nc.vector.tensor_scalar_mul(c[:], ps[:], 1025.0)
                nc.vector.tensor_sub(dd[:], c[:], ps[:])
                nc.vector.tensor_sub(dst, c[:], dd[:])

            for jc in range(2):
                csl = slice(jc * 512, (jc + 1) * 512)
                for a in range(8):
                    ps = pps.tile([128, 512], F32, tag="psp")
                    for d in range(8):
                        nc.tensor.matmul(
                            ps[:],
                            m2[d][:, a * 128 : (a + 1) * 128],
                            XT[d][:, csl],
                            start=(d == 0),
                            stop=(d == 7),
                        )
                    round13(UT[a][:, csl], ps)

        # natural-orientation bf16 x rows (for P@x) and Wv (for PX@Wv):
        # first needed once attention's first softmax lands (~45us in), so
        # issued after the U phase to keep startup HBM bandwidth for XT+m2.
        for j in range(16):
            nc.scalar.dma_start(XB[j][:], xb_d[j * 128 : (j + 1) * 128, :])
        for d in range(8):
            nc.scalar.dma_start(WV[d][:], wv_d[d * 128 : (d + 1) * 128, :])

        # ---- Attention per local row-block, software-pipelined -----------
        # PSUM banks (8): psS 2, psA 2 (PX and O accumulators time-share the
        # same banks: PX -> read out -> O -> read out, enforced by pool
        # rotation), psT 2, psTx 2.
        with (
            tc.tile_pool(name="psS", bufs=2, space="PSUM") as psS,
            tc.tile_pool(name="psA", bufs=1, space="PSUM") as psA,
            tc.tile_pool(name="psT", bufs=2, space="PSUM") as psT,
            tc.tile_pool(name="psTx", bufs=2, space="PSUM") as psTx,
            tc.tile_pool(name="attn", bufs=2) as pa,
            tc.tile_pool(name="attn1", bufs=2) as pa1,
            tc.tile_pool(name="pxsb", bufs=2) as ppx,
        ):
            # stage state carried from score/softmax stage to PX stage
            state = {}

            def emit_scores(l):
                chunks = CHUNKS[l]
                W = len(chunks) * 512
                lsl = slice(l * 128, (l + 1) * 128)
                S_sb = pa.tile([128, 2048], F32, tag="S")
                for k, ch in enumerate(chunks):
                    ps = psS.tile([128, 512], F32, tag="ps")
                    csl = slice(ch * 512, (ch + 1) * 512)
                    for ac in range(8):
                        nc.tensor.matmul(
                            ps[:],
                            UT[ac][:, lsl],
                            XT[ac][:, csl],
                            start=(ac == 0),
                            stop=(ac == 7),
                        )
                    mk = pa1.tile([128, 512], BF16, tag="mk")
                    nc.gpsimd.dma_start(mk[:], mask_d[lsl, csl])
                    nc.vector.tensor_add(S_sb[:, k * 512 : (k + 1) * 512], ps[:], mk[:])

                mx = pa1.tile([128, 1], F32, tag="mx")
                nc.vector.reduce_max(mx[:], S_sb[:, 0:W], axis=mybir.AxisListType.X)
                negb = pa1.tile([128, 1], F32, tag="negb")
                nc.vector.tensor_scalar_mul(negb[:], mx[:], -1.0 / 32.0)
                P_sb = pa.tile([128, 2048], BF16, tag="P")
                rs = pa1.tile([128, 1], F32, tag="rs")
                nc.scalar.activation(
                    P_sb[:, 0:W],
                    S_sb[:, 0:W],
                    mybir.ActivationFunctionType.Exp,
                    bias=negb[:],
                    scale=1.0 / 32.0,
                    accum_out=rs[:],
                )
                state[l] = (P_sb, rs)

            def emit_px(l):
                chunks = CHUNKS[l]
                lsl = slice(l * 128, (l + 1) * 128)
                P_sb, rs = state.pop(l)

                # PX = P @ x  (accumulate over unmasked 128-col blocks)
                pxacc = [psA.tile([128, 512], F32, name=f"px{h}", tag=f"a{h}") for h in range(2)]
                blocks = PVBLK[l]
                for i, q in enumerate(blocks):
                    vj = chunks[q // 4] * 4 + (q % 4)
                    pst = psT.tile([128, 128], BF16, tag="pst")
                    nc.tensor.transpose(
                        pst[:], P_sb[:, q * 128 : (q + 1) * 128], identb[:]
                    )
                    pt = pa1.tile([128, 128], BF16, tag="pt")
                    nc.vector.tensor_copy(pt[:], pst[:])
                    for half in range(2):
                        nc.tensor.matmul(
                            pxacc[half][:],
                            pt[:],
                            XB[vj][:, half * 512 : (half + 1) * 512],
                            start=(i == 0),
                            stop=(i == len(blocks) - 1),
                        )

                # PX -> SBUF, transpose to PX^T, project: out = PX @ Wv
                px = ppx.tile([128, 1024], F32, tag="px")
                for half in range(2):
                    nc.vector.tensor_copy(
                        px[:, half * 512 : (half + 1) * 512], pxacc[half][:]
                    )
                # transpose PX 128-col blocks and project each against Wv;
                # interleaving O matmuls with transposes keeps the single-
                # buffer transpose bank from stalling the PE.
                oacc = [psA.tile([128, 512], F32, name=f"oacc{h}", tag=f"a{h}") for h in range(2)]
                for dc in range(8):
                    pstx = psTx.tile([128, 128], F32, tag="pstx")
                    nc.tensor.transpose(
                        pstx[:], px[:, dc * 128 : (dc + 1) * 128], identf[:]
                    )
                    t = pa1.tile([128, 128], F32R, tag="pxt")
                    nc.vector.tensor_copy(t[:], pstx[:])
                    for half in range(2):
                        nc.tensor.matmul(
                            oacc[half][:],
                            t[:],
                            WV[dc][:, half * 512 : (half + 1) * 512],
                            start=(dc == 0),
                            stop=(dc == 7),
                        )

                rec = pa1.tile([128, 1], F32, tag="rec")
                nc.vector.reciprocal(rec[:], rs[:])
                for half in range(2):
                    o_sb = pa1.tile([128, 512], F32, tag="o")
                    nc.vector.tensor_scalar_mul(o_sb[:], oacc[half][:], rec[:])
                    nc.sync.dma_start(
                        out_d[lsl, half * 512 : (half + 1) * 512],
                        o_sb[:],
                    )

            # f32 identity for the PX^T transposes
            identf = pa.tile([128, 128], F32, tag="identf")
            masks.make_identity(nc, identf[:])

            for l in range(9):
                if l < 8:
                    emit_scores(l)
                if l >= 1:
                    emit_px(l - 1)

    nc.compile()
    _CACHE["nc"] = nc
    return nc


def _rtn22(a):
    """Round fp32 to fp22 (e8m13) with round-to-nearest on host.  The PE
    reads f32r operands truncated to fp22; pre-rounding makes that read
    lossless and replaces truncation bias with unbiased RTN error."""
    u = np.ascontiguousarray(a, dtype=np.float32).view(np.uint32)
    u = (u + np.uint32(0x200)) & np.uint32(0xFFFFFC00)
    return u.view(np.float32)


def _core_inputs(x, Wq, Wk, Wv, c):
    b = c // 2
    my = ABLK if c % 2 == 0 else BBLK
    perm = _perm_rows(my)
    gi = np.concatenate([np.arange(g * 128, (g + 1) * 128) for g in my])
    mask = np.where(perm[None, :] <= gi[:, None] + 1, 0.0, NEG).astype(
        ml_dtypes.bfloat16
    )
    key = ("m2", id(Wq), id(Wk))
    if _CACHE.get("m2key") != key:
        _CACHE["m2"] = _rtn22(
            (Wq.astype(np.float64) @ Wk.T.astype(np.float64)).astype(np.float32)
        )
        _CACHE["m2key"] = key
    xp = x[b][perm]
    return {
        "xt_perm": _rtn22(np.ascontiguousarray(xp.T)),
        "m2": _CACHE["m2"],
        "xb_perm": np.ascontiguousarray(xp).astype(ml_dtypes.bfloat16),
        "wv": _rtn22(Wv),
        "maskb": mask,
    }, (b, my)


def kernel(x, Wq, Wk, Wv):
    x = np.ascontiguousarray(np.asarray(x, dtype=np.float32))
    Wq = np.ascontiguousarray(np.asarray(Wq, dtype=np.float32))
    Wk = np.ascontiguousarray(np.asarray(Wk, dtype=np.float32))
    Wv = np.ascontiguousarray(np.asarray(Wv, dtype=np.float32))

    nc = _build()

    in_maps = []
    metas = []
    for c in range(NCORES):
        m, meta = _core_inputs(x, Wq, Wk, Wv, c)
        in_maps.append(m)
        metas.append(meta)

    res = run_bass_kernel_spmd(nc, in_maps, list(range(NCORES)))

    out = np.empty((B, S, DA), dtype=np.float32)
    for c in range(NCORES):
        b, my = metas[c]
        o = res.results[c]["out"]
        for l, g in enumerate(my):
            out[b, g * 128 : (g + 1) * 128] = o[l * 128 : (l + 1) * 128]
    return out


# revision 21
# speedup vs baseline: 1.2484x; 1.1349x over previous
"""Causal attention (single head, d=1024) on 8 trn2 NeuronCores.

Problem: x[4,2048,1024], Wq/Wk/Wv[1024,1024] fp32;
out = softmax(mask(QK^T)/sqrt(1024)) @ V with mask j <= i+1.

Sharding: 2 cores per batch. Causal row work grows ~linearly with row
index, so the two cores split the 16 row-blocks of 128 as
{g : g%4 in {0,3}} vs {g : g%4 in {1,2}} (balanced). Each core receives
x[b]^T with its own rows' columns permuted to the front so that every
core runs the same SPMD program; causality is enforced by a per-core
additive mask tensor (data, not code).

Key algebraic restructure: S = (x Wq)(x Wk)^T = x (Wq Wk^T) x^T.
M2 = Wq Wk^T is batch-independent and computed on the HOST, so the
device never computes K at all: U = x_own @ M2 (one Q-sized projection),
then S = U @ x^T against the resident x^T. This removes the whole
K-projection phase (2048x1024x1024 MACs per core).

Precision: U/S matmuls run as single-pass float32r (PE truncates reads
to fp22 = e8m13, fp32 accumulate; 1 cycle/row for moving dim >= 512).
Host inputs are pre-rounded to fp22 RTN; U is rounded to fp22 RTN on
device via a Veltkamp split so the PE read is lossless. V and P are
bf16 (output budget is lenient). Measured end-to-end relative error
~7e-3 against the fp32 reference (gate 2e-2).

Structure: x^T (8 MB) is DMA'd once into resident SBUF tiles in 512-col
chunks; V then U projections read it in place; attention row-blocks run
last, software-pipelined so softmax of block l overlaps score matmuls
of block l+1, with fully-masked 128-col P blocks skipped in P@V.
"""

import numpy as np
import ml_dtypes

import concourse.bass as bass
import concourse.mybir as mybir
import concourse.tile as tile
from concourse import bacc, masks
from concourse.bass_utils import run_bass_kernel_spmd

B, S, D, DA = 4, 2048, 1024, 1024
NCORES = 8
NBLK = S // 128  # 16 row blocks per batch
F32 = mybir.dt.float32
F32R = mybir.dt.float32r
BF16 = mybir.dt.bfloat16

ABLK = [g for g in range(NBLK) if g % 4 in (0, 3)]
BBLK = [g for g in range(NBLK) if g % 4 in (1, 2)]

NEG = -1e30


def _perm_rows(my):
    oth = [g for g in range(NBLK) if g not in my]
    idx = []
    for g in my + oth:
        idx.extend(range(g * 128, (g + 1) * 128))
    return np.array(idx, dtype=np.int64)


def _chunk_schedule():
    """Per local row-block l: which 512-col chunks of the permuted S row
    must be computed (union over the two roles, so the program is SPMD)."""
    sched = []
    for l in range(8):
        need = [False] * 4
        for my in (ABLK, BBLK):
            perm = _perm_rows(my)  # permuted col -> global row
            jmax = my[l] * 128 + 127 + 1  # max attended global col
            attended = perm <= jmax
            for ch in range(4):
                if attended[ch * 512 : (ch + 1) * 512].any():
                    need[ch] = True
        sched.append([ch for ch in range(4) if need[ch]])
    return sched


CHUNKS = _chunk_schedule()


def _pv_schedule():
    """Per local row-block l: which packed 128-col blocks of P (positions
    within the packed CHUNKS[l] layout) have any unmasked column for either
    role (union -> SPMD).  Blocks that are fully masked produce P == 0 and
    can be skipped in the P@V accumulation."""
    out = []
    for l in range(8):
        chunks = CHUNKS[l]
        needset = set()
        for my in (ABLK, BBLK):
            perm = _perm_rows(my)
            jmax = my[l] * 128 + 127 + 1
            attended = perm <= jmax
            for k, ch in enumerate(chunks):
                for q in range(4):
                    blk = ch * 4 + q
                    if attended[blk * 128 : (blk + 1) * 128].any():
                        needset.add(k * 4 + q)
        out.append(sorted(needset))
    return out


PVBLK = _pv_schedule()

_CACHE = {}


def _build():
    if "nc" in _CACHE:
        return _CACHE["nc"]

    nc = bacc.Bacc()
    # Inputs declared float32r (bit-identical to f32) so non-casting DMA
    # queues can load them.
    xt_d = nc.dram_tensor("xt_perm", [D, S], F32R, kind="ExternalInput")
    m2_d = nc.dram_tensor("m2", [D, D], F32R, kind="ExternalInput")
    wv_d = nc.dram_tensor("wv", [D, DA], F32R, kind="ExternalInput")
    mask_d = nc.dram_tensor("maskb", [1024, S], BF16, kind="ExternalInput")
    out_d = nc.dram_tensor("out", [1024, DA], F32, kind="ExternalOutput")

    from contextlib import ExitStack

    with tile.TileContext(nc) as tc, ExitStack() as stack:
        cpool = stack.enter_context(tc.tile_pool(name="const", bufs=1))
        identb = cpool.tile([128, 128], BF16, tag="identb")
        masks.make_identity(nc, identb[:])

        # long-lived residents (live until the end of attention)
        vpool = stack.enter_context(tc.tile_pool(name="vres", bufs=1))
        V = [vpool.tile([128, DA], BF16, name=f"v{j}", tag=f"v{j}") for j in range(16)]
        upool = stack.enter_context(tc.tile_pool(name="utres", bufs=1))
        UT = [upool.tile([128, 1024], F32R, name=f"ut{a}", tag=f"ut{a}") for a in range(8)]
        xpool = stack.enter_context(tc.tile_pool(name="xtres", bufs=1))
        XT = [xpool.tile([128, S], F32R, name=f"xt{d}", tag=f"xt{d}") for d in range(8)]

        # x^T loaded once, in 512-col chunks so consumers start early
        for jc in range(4):
            for d in range(8):
                nc.sync.dma_start(
                    XT[d][:, jc * 512 : (jc + 1) * 512],
                    xt_d[d * 128 : (d + 1) * 128, jc * 512 : (jc + 1) * 512],
                )

        # ---- Projections: V (all rows) then U = x_own @ M2 ---------------
        with (
            tc.tile_pool(name="wproj", bufs=1) as pw,
            tc.tile_pool(name="m2w", bufs=1) as pm,
            tc.tile_pool(name="vtmp", bufs=2) as ptmp,
            tc.tile_pool(name="psproj", bufs=4, space="PSUM") as pps,
        ):
            wv = [pw.tile([128, DA], F32R, name=f"wv{d}", tag=f"wv{d}") for d in range(8)]
            m2 = [pm.tile([128, D], F32R, name=f"m2{d}", tag=f"m2{d}") for d in range(8)]
            # wv halves in consumption order: the first V psum group only
            # needs [:, 0:512] of every wv tile, so it starts ~6us earlier.
            for half in range(2):
                csl = slice(half * 512, (half + 1) * 512)
                for d in range(8):
                    nc.gpsimd.dma_start(
                        wv[d][:, csl], wv_d[d * 128 : (d + 1) * 128, csl]
                    )

            def round13(dst, ps):
                # Veltkamp split: round PSUM fp32 to 14-bit significand
                # (e8m13) round-to-nearest, so the PE's f32r read of dst is
                # lossless.
                c = ptmp.tile([128, 512], F32, tag="vc")
                dd = ptmp.tile([128, 512], F32, tag="vd")
                nc.vector.tensor_scalar_mul(c[:], ps[:], 1025.0)
                nc.vector.tensor_sub(dd[:], c[:], ps[:])
                nc.vector.tensor_sub(dst, c[:], dd[:])

            # V (all rows)
            for jc in range(4):
                for q in range(4):
                    vj = jc * 4 + q
                    jsl = slice(vj * 128, (vj + 1) * 128)
                    for half in range(2):
                        ps = pps.tile([128, 512], F32, tag="psp")
                        for d in range(8):
                            nc.tensor.matmul(
                                ps[:],
                                XT[d][:, jsl],
                                wv[d][:, half * 512 : (half + 1) * 512],
                                start=(d == 0),
                                stop=(d == 7),
                            )
                        nc.vector.tensor_copy(
                            V[vj][:, half * 512 : (half + 1) * 512], ps[:]
                        )

            # m2 loads issued after the V-phase emission so startup HBM
            # bandwidth goes to wv + the first x^T chunks; m2 arrives well
            # before the U phase consumes it.
            for d in range(8):
                nc.scalar.dma_start(m2[d][:], m2_d[d * 128 : (d + 1) * 128, :])

            # U^T = M2^T x_own^T (own rows = first two chunks)
            for jc in range(2):
                csl = slice(jc * 512, (jc + 1) * 512)
                for a in range(8):
                    ps = pps.tile([128, 512], F32, tag="psp")
                    for d in range(8):
                        nc.tensor.matmul(
                            ps[:],
                            m2[d][:, a * 128 : (a + 1) * 128],
                            XT[d][:, csl],
                            start=(d == 0),
                            stop=(d == 7),
                        )
                    round13(UT[a][:, csl], ps)

        # ---- Attention per local row-block, software-pipelined -----------
        with (
            tc.tile_pool(name="attn", bufs=2) as pa,
            tc.tile_pool(name="attn1", bufs=2) as pa1,
            tc.tile_pool(name="psS", bufs=2, space="PSUM") as psS,
            tc.tile_pool(name="psT", bufs=2, space="PSUM") as psT,
            tc.tile_pool(name="psO", bufs=2, space="PSUM") as psO,
        ):
            # stage state carried from score/softmax stage to PV stage
            state = {}

            def emit_scores(l):
                chunks = CHUNKS[l]
                W = len(chunks) * 512
                lsl = slice(l * 128, (l + 1) * 128)
                S_sb = pa.tile([128, 2048], F32, tag="S")
                for k, ch in enumerate(chunks):
                    ps = psS.tile([128, 512], F32, tag="ps")
                    csl = slice(ch * 512, (ch + 1) * 512)
                    for ac in range(8):
                        nc.tensor.matmul(
                            ps[:],
                            UT[ac][:, lsl],
                            XT[ac][:, csl],
                            start=(ac == 0),
                            stop=(ac == 7),
                        )
                    mk = pa1.tile([128, 512], BF16, tag="mk")
                    nc.gpsimd.dma_start(mk[:], mask_d[lsl, csl])
                    nc.vector.tensor_add(S_sb[:, k * 512 : (k + 1) * 512], ps[:], mk[:])

                mx = pa1.tile([128, 1], F32, tag="mx")
                nc.vector.reduce_max(mx[:], S_sb[:, 0:W], axis=mybir.AxisListType.X)
                negb = pa1.tile([128, 1], F32, tag="negb")
                nc.vector.tensor_scalar_mul(negb[:], mx[:], -1.0 / 32.0)
                P_sb = pa.tile([128, 2048], BF16, tag="P")
                rs = pa1.tile([128, 1], F32, tag="rs")
                nc.scalar.activation(
                    P_sb[:, 0:W],
                    S_sb[:, 0:W],
                    mybir.ActivationFunctionType.Exp,
                    bias=negb[:],
                    scale=1.0 / 32.0,
                    accum_out=rs[:],
                )
                state[l] = (P_sb, rs)

            def emit_pv(l):
                chunks = CHUNKS[l]
                lsl = slice(l * 128, (l + 1) * 128)
                P_sb, rs = state.pop(l)
                oacc = [psO.tile([128, 512], F32, name=f"oacc{h}", tag=f"oacc{h}") for h in range(2)]
                blocks = PVBLK[l]
                for i, q in enumerate(blocks):
                    vj = chunks[q // 4] * 4 + (q % 4)
                    pst = psT.tile([128, 128], BF16, tag="pst")
                    nc.tensor.transpose(
                        pst[:], P_sb[:, q * 128 : (q + 1) * 128], identb[:]
                    )
                    pt = pa1.tile([128, 128], BF16, tag="pt")
                    nc.vector.tensor_copy(pt[:], pst[:])
                    for half in range(2):
                        nc.tensor.matmul(
                            oacc[half][:],
                            pt[:],
                            V[vj][:, half * 512 : (half + 1) * 512],
                            start=(i == 0),
                            stop=(i == len(blocks) - 1),
                        )

                rec = pa1.tile([128, 1], F32, tag="rec")
                nc.vector.reciprocal(rec[:], rs[:])
                for half in range(2):
                    o_sb = pa1.tile([128, 512], F32, tag="o")
                    nc.vector.tensor_scalar_mul(o_sb[:], oacc[half][:], rec[:])
                    nc.sync.dma_start(
                        out_d[lsl, half * 512 : (half + 1) * 512],
                        o_sb[:],
                    )

            for l in range(9):
                if l < 8:
                    emit_scores(l)
                if l >= 1:
                    emit_pv(l - 1)

    nc.compile()
    _CACHE["nc"] = nc
    return nc


def _rtn22(a):
    """Round fp32 to fp22 (e8m13) with round-to-nearest on host.  The PE
    reads f32r operands truncated to fp22; pre-rounding makes that read
    lossless and replaces truncation bias with unbiased RTN error."""
    u = np.ascontiguousarray(a, dtype=np.float32).view(np.uint32)
    u = (u + np.uint32(0x200)) & np.uint32(0xFFFFFC00)
    return u.view(np.float32)


def _core_inputs(x, Wq, Wk, Wv, c):
    b = c // 2
    my = ABLK if c % 2 == 0 else BBLK
    perm = _perm_rows(my)
    gi = np.concatenate([np.arange(g * 128, (g + 1) * 128) for g in my])
    mask = np.where(perm[None, :] <= gi[:, None] + 1, 0.0, NEG).astype(
        ml_dtypes.bfloat16
    )
    key = ("m2", id(Wq), id(Wk))
    if _CACHE.get("m2key") != key:
        _CACHE["m2"] = _rtn22(
            (Wq.astype(np.float64) @ Wk.T.astype(np.float64)).astype(np.float32)
        )
        _CACHE["m2key"] = key
    return {
        "xt_perm": _rtn22(np.ascontiguousarray(x[b].T[:, perm])),
        "m2": _CACHE["m2"],
        "wv": _rtn22(Wv),
        "maskb": mask,
    }, (b, my)


def kernel(x, Wq, Wk, Wv):
    x = np.ascontiguousarray(np.asarray(x, dtype=np.float32))
    Wq = np.ascontiguousarray(np.asarray(Wq, dtype=np.float32))
    Wk = np.ascontiguousarray(np.asarray(Wk, dtype=np.float32))
    Wv = np.ascontiguousarray(np.asarray(Wv, dtype=np.float32))

    nc = _build()

    in_maps = []
    metas = []
    for c in range(NCORES):
        m, meta = _core_inputs(x, Wq, Wk, Wv, c)
        in_maps.append(m)
        metas.append(meta)

    res = run_bass_kernel_spmd(nc, in_maps, list(range(NCORES)))

    out = np.empty((B, S, DA), dtype=np.float32)
    for c in range(NCORES):
        b, my = metas[c]
        o = res.results[c]["out"]
        for l, g in enumerate(my):
            out[b, g * 128 : (g + 1) * 128] = o[l * 128 : (l + 1) * 128]
    return out


# revision 23
# speedup vs baseline: 1.2707x; 1.0179x over previous
"""Causal attention (single head, d=1024) on 8 trn2 NeuronCores.

Problem: x[4,2048,1024], Wq/Wk/Wv[1024,1024] fp32;
out = softmax(mask(QK^T)/sqrt(1024)) @ V with mask j <= i+1.

Sharding: 2 cores per batch. Causal row work grows ~linearly with row
index, so the two cores split the 16 row-blocks of 128 as
{g : g%4 in {0,3}} vs {g : g%4 in {1,2}} (balanced). Each core receives
x[b]^T with its own rows' columns permuted to the front so that every
core runs the same SPMD program; causality is enforced by a per-core
additive mask tensor (data, not code).

Key algebraic restructure: S = (x Wq)(x Wk)^T = x (Wq Wk^T) x^T.
M2 = Wq Wk^T is batch-independent and computed on the HOST, so the
device never computes K at all: U = x_own @ M2 (one Q-sized projection),
then S = U @ x^T against the resident x^T. This removes the whole
K-projection phase (2048x1024x1024 MACs per core).

Precision: U/S matmuls run as single-pass float32r (PE truncates reads
to fp22 = e8m13, fp32 accumulate; 1 cycle/row for moving dim >= 512).
Host inputs are pre-rounded to fp22 RTN; U is rounded to fp22 RTN on
device via a Veltkamp split so the PE read is lossless. V and P are
bf16 (output budget is lenient). Measured end-to-end relative error
~7e-3 against the fp32 reference (gate 2e-2).

Structure: x^T (8 MB) is DMA'd once into resident SBUF tiles in 512-col
chunks; V then U projections read it in place; attention row-blocks run
last, software-pipelined so softmax of block l overlaps score matmuls
of block l+1, with fully-masked 128-col P blocks skipped in P@V.
"""

import numpy as np
import ml_dtypes

import concourse.bass as bass
import concourse.mybir as mybir
import concourse.tile as tile
from concourse import bacc, masks
from concourse.bass_utils import run_bass_kernel_spmd

B, S, D, DA = 4, 2048, 1024, 1024
NCORES = 8
NBLK = S // 128  # 16 row blocks per batch
F32 = mybir.dt.float32
F32R = mybir.dt.float32r
BF16 = mybir.dt.bfloat16

ABLK = [g for g in range(NBLK) if g % 4 in (0, 3)]
BBLK = [g for g in range(NBLK) if g % 4 in (1, 2)]

NEG = -1e30


def _perm_rows(my):
    oth = [g for g in range(NBLK) if g not in my]
    idx = []
    for g in my + oth:
        idx.extend(range(g * 128, (g + 1) * 128))
    return np.array(idx, dtype=np.int64)


def _chunk_schedule():
    """Per local row-block l: which 512-col chunks of the permuted S row
    must be computed (union over the two roles, so the program is SPMD)."""
    sched = []
    for l in range(8):
        need = [False] * 4
        for my in (ABLK, BBLK):
            perm = _perm_rows(my)  # permuted col -> global row
            jmax = my[l] * 128 + 127 + 1  # max attended global col
            attended = perm <= jmax
            for ch in range(4):
                if attended[ch * 512 : (ch + 1) * 512].any():
                    need[ch] = True
        sched.append([ch for ch in range(4) if need[ch]])
    return sched


CHUNKS = _chunk_schedule()


def _pv_schedule():
    """Per local row-block l: which packed 128-col blocks of P (positions
    within the packed CHUNKS[l] layout) have any unmasked column for either
    role (union -> SPMD).  Blocks that are fully masked produce P == 0 and
    can be skipped in the P@V accumulation."""
    out = []
    for l in range(8):
        chunks = CHUNKS[l]
        needset = set()
        for my in (ABLK, BBLK):
            perm = _perm_rows(my)
            jmax = my[l] * 128 + 127 + 1
            attended = perm <= jmax
            for k, ch in enumerate(chunks):
                for q in range(4):
                    blk = ch * 4 + q
                    if attended[blk * 128 : (blk + 1) * 128].any():
                        needset.add(k * 4 + q)
        out.append(sorted(needset))
    return out


PVBLK = _pv_schedule()

_CACHE = {}


def _build():
    if "nc" in _CACHE:
        return _CACHE["nc"]

    nc = bacc.Bacc()
    # Inputs declared float32r (bit-identical to f32) so non-casting DMA
    # queues can load them.
    xt_d = nc.dram_tensor("xt_perm", [D, S], F32R, kind="ExternalInput")
    m2_d = nc.dram_tensor("m2", [D, D], F32R, kind="ExternalInput")
    wv_d = nc.dram_tensor("wv", [D, DA], F32R, kind="ExternalInput")
    mask_d = nc.dram_tensor("maskb", [1024, S], BF16, kind="ExternalInput")
    out_d = nc.dram_tensor("out", [1024, DA], F32, kind="ExternalOutput")

    from contextlib import ExitStack

    with tile.TileContext(nc) as tc, ExitStack() as stack:
        cpool = stack.enter_context(tc.tile_pool(name="const", bufs=1))
        identb = cpool.tile([128, 128], BF16, tag="identb")
        masks.make_identity(nc, identb[:])

        # long-lived residents (live until the end of attention)
        vpool = stack.enter_context(tc.tile_pool(name="vres", bufs=1))
        V = [vpool.tile([128, DA], BF16, name=f"v{j}", tag=f"v{j}") for j in range(16)]
        upool = stack.enter_context(tc.tile_pool(name="utres", bufs=1))
        UT = [upool.tile([128, 1024], F32R, name=f"ut{a}", tag=f"ut{a}") for a in range(8)]
        xpool = stack.enter_context(tc.tile_pool(name="xtres", bufs=1))
        XT = [xpool.tile([128, S], F32R, name=f"xt{d}", tag=f"xt{d}") for d in range(8)]

        # x^T loaded once, in 512-col chunks so consumers start early
        for jc in range(4):
            for d in range(8):
                nc.sync.dma_start(
                    XT[d][:, jc * 512 : (jc + 1) * 512],
                    xt_d[d * 128 : (d + 1) * 128, jc * 512 : (jc + 1) * 512],
                )

        # ---- Projections: V (all rows) then U = x_own @ M2 ---------------
        with (
            tc.tile_pool(name="wproj", bufs=1) as pw,
            tc.tile_pool(name="m2w", bufs=1) as pm,
            tc.tile_pool(name="vtmp", bufs=2) as ptmp,
            tc.tile_pool(name="psproj", bufs=4, space="PSUM") as pps,
        ):
            wv = [pw.tile([128, DA], F32R, name=f"wv{d}", tag=f"wv{d}") for d in range(8)]
            m2 = [pm.tile([128, D], F32R, name=f"m2{d}", tag=f"m2{d}") for d in range(8)]
            for d in range(8):
                nc.gpsimd.dma_start(wv[d][:], wv_d[d * 128 : (d + 1) * 128, :])
                nc.scalar.dma_start(m2[d][:], m2_d[d * 128 : (d + 1) * 128, :])

            def round13(dst, ps):
                # Veltkamp split: round PSUM fp32 to 14-bit significand
                # (e8m13) round-to-nearest, so the PE's f32r read of dst is
                # lossless.
                c = ptmp.tile([128, 512], F32, tag="vc")
                dd = ptmp.tile([128, 512], F32, tag="vd")
                nc.vector.tensor_scalar_mul(c[:], ps[:], 1025.0)
                nc.vector.tensor_sub(dd[:], c[:], ps[:])
                nc.vector.tensor_sub(dst, c[:], dd[:])

            # V (all rows)
            for jc in range(4):
                for q in range(4):
                    vj = jc * 4 + q
                    jsl = slice(vj * 128, (vj + 1) * 128)
                    for half in range(2):
                        ps = pps.tile([128, 512], F32, tag="psp")
                        for d in range(8):
                            nc.tensor.matmul(
                                ps[:],
                                XT[d][:, jsl],
                                wv[d][:, half * 512 : (half + 1) * 512],
                                start=(d == 0),
                                stop=(d == 7),
                            )
                        nc.vector.tensor_copy(
                            V[vj][:, half * 512 : (half + 1) * 512], ps[:]
                        )

            # U^T = M2^T x_own^T (own rows = first two chunks)
            for jc in range(2):
                csl = slice(jc * 512, (jc + 1) * 512)
                for a in range(8):
                    ps = pps.tile([128, 512], F32, tag="psp")
                    for d in range(8):
                        nc.tensor.matmul(
                            ps[:],
                            m2[d][:, a * 128 : (a + 1) * 128],
                            XT[d][:, csl],
                            start=(d == 0),
                            stop=(d == 7),
                        )
                    round13(UT[a][:, csl], ps)

        # ---- Attention per local row-block, software-pipelined -----------
        with (
            tc.tile_pool(name="attn", bufs=2) as pa,
            tc.tile_pool(name="attn1", bufs=2) as pa1,
            tc.tile_pool(name="psS", bufs=2, space="PSUM") as psS,
            tc.tile_pool(name="psT", bufs=2, space="PSUM") as psT,
            tc.tile_pool(name="psO", bufs=2, space="PSUM") as psO,
        ):
            # stage state carried from score/softmax stage to PV stage
            state = {}

            def emit_scores(l):
                chunks = CHUNKS[l]
                W = len(chunks) * 512
                lsl = slice(l * 128, (l + 1) * 128)
                S_sb = pa.tile([128, 2048], F32, tag="S")
                for k, ch in enumerate(chunks):
                    ps = psS.tile([128, 512], F32, tag="ps")
                    csl = slice(ch * 512, (ch + 1) * 512)
                    for ac in range(8):
                        nc.tensor.matmul(
                            ps[:],
                            UT[ac][:, lsl],
                            XT[ac][:, csl],
                            start=(ac == 0),
                            stop=(ac == 7),
                        )
                    mk = pa1.tile([128, 512], BF16, tag="mk")
                    nc.gpsimd.dma_start(mk[:], mask_d[lsl, csl])
                    nc.vector.tensor_add(S_sb[:, k * 512 : (k + 1) * 512], ps[:], mk[:])

                mx = pa1.tile([128, 1], F32, tag="mx")
                nc.vector.reduce_max(mx[:], S_sb[:, 0:W], axis=mybir.AxisListType.X)
                negb = pa1.tile([128, 1], F32, tag="negb")
                nc.vector.tensor_scalar_mul(negb[:], mx[:], -1.0 / 32.0)
                P_sb = pa.tile([128, 2048], BF16, tag="P")
                rs = pa1.tile([128, 1], F32, tag="rs")
                nc.scalar.activation(
                    P_sb[:, 0:W],
                    S_sb[:, 0:W],
                    mybir.ActivationFunctionType.Exp,
                    bias=negb[:],
                    scale=1.0 / 32.0,
                    accum_out=rs[:],
                )
                state[l] = (P_sb, rs)

            def emit_pv(l):
                chunks = CHUNKS[l]
                lsl = slice(l * 128, (l + 1) * 128)
                P_sb, rs = state.pop(l)
                oacc = [psO.tile([128, 512], F32, name=f"oacc{h}", tag=f"oacc{h}") for h in range(2)]
                blocks = PVBLK[l]
                for i, q in enumerate(blocks):
                    vj = chunks[q // 4] * 4 + (q % 4)
                    pst = psT.tile([128, 128], BF16, tag="pst")
                    nc.tensor.transpose(
                        pst[:], P_sb[:, q * 128 : (q + 1) * 128], identb[:]
                    )
                    pt = pa1.tile([128, 128], BF16, tag="pt")
                    nc.vector.tensor_copy(pt[:], pst[:])
                    for half in range(2):
                        nc.tensor.matmul(
                            oacc[half][:],
                            pt[:],
                            V[vj][:, half * 512 : (half + 1) * 512],
                            start=(i == 0),
                            stop=(i == len(blocks) - 1),
                        )

                rec = pa1.tile([128, 1], F32, tag="rec")
                nc.vector.reciprocal(rec[:], rs[:])
                for half in range(2):
                    o_sb = pa1.tile([128, 512], F32, tag="o")
                    nc.vector.tensor_scalar_mul(o_sb[:], oacc[half][:], rec[:])
                    nc.sync.dma_start(
                        out_d[lsl, half * 512 : (half + 1) * 512],
                        o_sb[:],
                    )

            for l in range(9):
                if l < 8:
                    emit_scores(l)
                if l >= 1:
                    emit_pv(l - 1)

    nc.compile()
    _CACHE["nc"] = nc
    return nc


def _rtn22(a):
    """Round fp32 to fp22 (e8m13) with round-to-nearest on host.  The PE
    reads f32r operands truncated to fp22; pre-rounding makes that read
    lossless and replaces truncation bias with unbiased RTN error."""
    u = np.ascontiguousarray(a, dtype=np.float32).view(np.uint32)
    u = (u + np.uint32(0x200)) & np.uint32(0xFFFFFC00)
    return u.view(np.float32)


def _core_inputs(x, Wq, Wk, Wv, c):
    b = c // 2
    my = ABLK if c % 2 == 0 else BBLK
    perm = _perm_rows(my)
    gi = np.concatenate([np.arange(g * 128, (g + 1) * 128) for g in my])
    mask = np.where(perm[None, :] <= gi[:, None] + 1, 0.0, NEG).astype(
        ml_dtypes.bfloat16
    )
    key = ("m2", id(Wq), id(Wk))
    if _CACHE.get("m2key") != key:
        _CACHE["m2"] = _rtn22(
            (Wq.astype(np.float64) @ Wk.T.astype(np.float64)).astype(np.float32)
        )
        _CACHE["m2key"] = key
    return {
        "xt_perm": _rtn22(np.ascontiguousarray(x[b].T[:, perm])),
        "m2": _CACHE["m2"],
        "wv": _rtn22(Wv),
        "maskb": mask,
    }, (b, my)


def kernel(x, Wq, Wk, Wv):
    x = np.ascontiguousarray(np.asarray(x, dtype=np.float32))
    Wq = np.ascontiguousarray(np.asarray(Wq, dtype=np.float32))
    Wk = np.ascontiguousarray(np.asarray(Wk, dtype=np.float32))
    Wv = np.ascontiguousarray(np.asarray(Wv, dtype=np.float32))

    # Recompute M2 unconditionally for this call's weights (the id-based
    # cache in _core_inputs then only dedups the 8 per-core calls below).
    _CACHE["m2"] = _rtn22(
        (Wq.astype(np.float64) @ Wk.T.astype(np.float64)).astype(np.float32)
    )
    _CACHE["m2key"] = ("m2", id(Wq), id(Wk))

    nc = _build()

    in_maps = []
    metas = []
    for c in range(NCORES):
        m, meta = _core_inputs(x, Wq, Wk, Wv, c)
        in_maps.append(m)
        metas.append(meta)

    res = run_bass_kernel_spmd(nc, in_maps, list(range(NCORES)))

    out = np.empty((B, S, DA), dtype=np.float32)
    for c in range(NCORES):
        b, my = metas[c]
        o = res.results[c]["out"]
        for l, g in enumerate(my):
            out[b, g * 128 : (g + 1) * 128] = o[l * 128 : (l + 1) * 128]
    return out


# revision 26
# speedup vs baseline: 1.2873x; 1.0131x over previous
"""Causal attention (single head, d=1024) on 8 trn2 NeuronCores.

Problem: x[4,2048,1024], Wq/Wk/Wv[1024,1024] fp32;
out = softmax(mask(QK^T)/sqrt(1024)) @ V with mask j <= i+1.

Sharding: 2 cores per batch. Causal row work grows ~linearly with row
index, so the two cores split the 16 row-blocks of 128 as
{g : g%4 in {0,3}} vs {g : g%4 in {1,2}} (balanced). Each core receives
x[b]^T with its own rows' columns permuted to the front so that every
core runs the same SPMD program; causality is enforced by a per-core
additive mask tensor (data, not code).

Key algebraic restructure: S = (x Wq)(x Wk)^T = x (Wq Wk^T) x^T.
M2 = Wq Wk^T is batch-independent and computed on the HOST, so the
device never computes K at all: U = x_own @ M2 (one Q-sized projection),
then S = U @ x^T against the resident x^T. This removes the whole
K-projection phase (2048x1024x1024 MACs per core).

Precision: U/S matmuls run as single-pass float32r (PE truncates reads
to fp22 = e8m13, fp32 accumulate; 1 cycle/row for moving dim >= 512).
Host inputs are pre-rounded to fp22 RTN; U is rounded to fp22 RTN on
device via a Veltkamp split so the PE read is lossless. V and P are
bf16 (output budget is lenient). Measured end-to-end relative error
~7e-3 against the fp32 reference (gate 2e-2).

Structure: x^T (8 MB) is DMA'd once into resident SBUF tiles in 512-col
chunks; V then U projections read it in place; attention row-blocks run
last, software-pipelined so softmax of block l overlaps score matmuls
of block l+1, with fully-masked 128-col P blocks skipped in P@V.
"""

import numpy as np
import ml_dtypes

import concourse.bass as bass
import concourse.mybir as mybir
import concourse.tile as tile
from concourse import bacc, masks
from concourse.bass_utils import run_bass_kernel_spmd

B, S, D, DA = 4, 2048, 1024, 1024
NCORES = 8
NBLK = S // 128  # 16 row blocks per batch
F32 = mybir.dt.float32
F32R = mybir.dt.float32r
BF16 = mybir.dt.bfloat16

ABLK = [g for g in range(NBLK) if g % 4 in (0, 3)]
BBLK = [g for g in range(NBLK) if g % 4 in (1, 2)]

NEG = -1e30


def _perm_rows(my):
    oth = [g for g in range(NBLK) if g not in my]
    idx = []
    for g in my + oth:
        idx.extend(range(g * 128, (g + 1) * 128))
    return np.array(idx, dtype=np.int64)


def _chunk_schedule():
    """Per local row-block l: which 512-col chunks of the permuted S row
    must be computed (union over the two roles, so the program is SPMD)."""
    sched = []
    for l in range(8):
        need = [False] * 4
        for my in (ABLK, BBLK):
            perm = _perm_rows(my)  # permuted col -> global row
            jmax = my[l] * 128 + 127 + 1  # max attended global col
            attended = perm <= jmax
            for ch in range(4):
                if attended[ch * 512 : (ch + 1) * 512].any():
                    need[ch] = True
        sched.append([ch for ch in range(4) if need[ch]])
    return sched


CHUNKS = _chunk_schedule()


def _pv_schedule():
    """Per local row-block l: which packed 128-col blocks of P (positions
    within the packed CHUNKS[l] layout) have any unmasked column for either
    role (union -> SPMD).  Blocks that are fully masked produce P == 0 and
    can be skipped in the P@V accumulation."""
    out = []
    for l in range(8):
        chunks = CHUNKS[l]
        needset = set()
        for my in (ABLK, BBLK):
            perm = _perm_rows(my)
            jmax = my[l] * 128 + 127 + 1
            attended = perm <= jmax
            for k, ch in enumerate(chunks):
                for q in range(4):
                    blk = ch * 4 + q
                    if attended[blk * 128 : (blk + 1) * 128].any():
                        needset.add(k * 4 + q)
        out.append(sorted(needset))
    return out


PVBLK = _pv_schedule()


def _score_widths():
    """Per (l, chunk k): moving width for the score matmul.  Within a chunk
    the needed 128-col blocks always form a prefix (union of two per-role
    prefixes), so the matmul can stop early; f32r needs moving >= 256 to
    stay at 1 cycle/row.  The trimmed tail is filled from the mask
    (-1e30 -> exp 0), keeping the packed P layout unchanged."""
    out = []
    for l in range(8):
        nch = len(CHUNKS[l])
        widths = []
        for k in range(nch):
            n = sum(1 for q in PVBLK[l] if k * 4 <= q < (k + 1) * 4)
            widths.append(min(512, max(256, 128 * n)))
        out.append(widths)
    return out


SWID = _score_widths()

_CACHE = {}


def _build():
    if "nc" in _CACHE:
        return _CACHE["nc"]

    nc = bacc.Bacc()
    # Inputs declared float32r (bit-identical to f32) so non-casting DMA
    # queues can load them.
    xt_d = nc.dram_tensor("xt_perm", [D, S], F32R, kind="ExternalInput")
    m2_d = nc.dram_tensor("m2", [D, D], F32R, kind="ExternalInput")
    wv_d = nc.dram_tensor("wv", [D, DA], F32R, kind="ExternalInput")
    mask_d = nc.dram_tensor("maskb", [1024, S], BF16, kind="ExternalInput")
    out_d = nc.dram_tensor("out", [1024, DA], F32, kind="ExternalOutput")

    from contextlib import ExitStack

    with tile.TileContext(nc) as tc, ExitStack() as stack:
        cpool = stack.enter_context(tc.tile_pool(name="const", bufs=1))
        identb = cpool.tile([128, 128], BF16, tag="identb")
        masks.make_identity(nc, identb[:])

        # long-lived residents (live until the end of attention)
        vpool = stack.enter_context(tc.tile_pool(name="vres", bufs=1))
        V = [vpool.tile([128, DA], BF16, name=f"v{j}", tag=f"v{j}") for j in range(16)]
        upool = stack.enter_context(tc.tile_pool(name="utres", bufs=1))
        UT = [upool.tile([128, 1024], F32R, name=f"ut{a}", tag=f"ut{a}") for a in range(8)]
        xpool = stack.enter_context(tc.tile_pool(name="xtres", bufs=1))
        XT = [xpool.tile([128, S], F32R, name=f"xt{d}", tag=f"xt{d}") for d in range(8)]

        # x^T loaded once, in 512-col chunks so consumers start early
        for jc in range(4):
            for d in range(8):
                nc.sync.dma_start(
                    XT[d][:, jc * 512 : (jc + 1) * 512],
                    xt_d[d * 128 : (d + 1) * 128, jc * 512 : (jc + 1) * 512],
                )

        # ---- Projections: V (all rows) then U = x_own @ M2 ---------------
        with (
            tc.tile_pool(name="wproj", bufs=1) as pw,
            tc.tile_pool(name="m2w", bufs=1) as pm,
            tc.tile_pool(name="vtmp", bufs=2) as ptmp,
            tc.tile_pool(name="psproj", bufs=4, space="PSUM") as pps,
        ):
            wv = [pw.tile([128, DA], F32R, name=f"wv{d}", tag=f"wv{d}") for d in range(8)]
            m2 = [pm.tile([128, D], F32R, name=f"m2{d}", tag=f"m2{d}") for d in range(8)]
            for d in range(8):
                nc.gpsimd.dma_start(wv[d][:], wv_d[d * 128 : (d + 1) * 128, :])
                nc.scalar.dma_start(m2[d][:], m2_d[d * 128 : (d + 1) * 128, :])

            def round13(dst, ps):
                # Veltkamp split: round PSUM fp32 to 14-bit significand
                # (e8m13) round-to-nearest, so the PE's f32r read of dst is
                # lossless.
                c = ptmp.tile([128, 512], F32, tag="vc")
                dd = ptmp.tile([128, 512], F32, tag="vd")
                nc.vector.tensor_scalar_mul(c[:], ps[:], 1025.0)
                nc.vector.tensor_sub(dd[:], c[:], ps[:])
                nc.vector.tensor_sub(dst, c[:], dd[:])

            # V (all rows)
            for jc in range(4):
                for q in range(4):
                    vj = jc * 4 + q
                    jsl = slice(vj * 128, (vj + 1) * 128)
                    for half in range(2):
                        ps = pps.tile([128, 512], F32, tag="psp")
                        for d in range(8):
                            nc.tensor.matmul(
                                ps[:],
                                XT[d][:, jsl],
                                wv[d][:, half * 512 : (half + 1) * 512],
                                start=(d == 0),
                                stop=(d == 7),
                            )
                        nc.vector.tensor_copy(
                            V[vj][:, half * 512 : (half + 1) * 512], ps[:]
                        )

            # U^T = M2^T x_own^T (own rows = first two chunks)
            for jc in range(2):
                csl = slice(jc * 512, (jc + 1) * 512)
                for a in range(8):
                    ps = pps.tile([128, 512], F32, tag="psp")
                    for d in range(8):
                        nc.tensor.matmul(
                            ps[:],
                            m2[d][:, a * 128 : (a + 1) * 128],
                            XT[d][:, csl],
                            start=(d == 0),
                            stop=(d == 7),
                        )
                    round13(UT[a][:, csl], ps)

        # ---- Attention per local row-block, software-pipelined -----------
        with (
            tc.tile_pool(name="attn", bufs=2) as pa,
            tc.tile_pool(name="attn1", bufs=2) as pa1,
            tc.tile_pool(name="psS", bufs=2, space="PSUM") as psS,
            tc.tile_pool(name="psT", bufs=2, space="PSUM") as psT,
            tc.tile_pool(name="psO", bufs=2, space="PSUM") as psO,
        ):
            # stage state carried from score/softmax stage to PV stage
            state = {}

            def emit_scores(l):
                chunks = CHUNKS[l]
                W = len(chunks) * 512
                lsl = slice(l * 128, (l + 1) * 128)
                S_sb = pa.tile([128, 2048], F32, tag="S")
                for k, ch in enumerate(chunks):
                    w = SWID[l][k]
                    ps = psS.tile([128, 512], F32, tag="ps")
                    csl = slice(ch * 512, ch * 512 + w)
                    for ac in range(8):
                        nc.tensor.matmul(
                            ps[:, 0:w],
                            UT[ac][:, lsl],
                            XT[ac][:, csl],
                            start=(ac == 0),
                            stop=(ac == 7),
                        )
                    mk = pa1.tile([128, 512], BF16, tag="mk")
                    nc.gpsimd.dma_start(mk[:], mask_d[lsl, ch * 512 : (ch + 1) * 512])
                    nc.vector.tensor_add(
                        S_sb[:, k * 512 : k * 512 + w], ps[:, 0:w], mk[:, 0:w]
                    )
                    if w < 512:
                        # fully-masked tail: pure mask (-1e30), exp -> 0
                        nc.vector.tensor_copy(
                            S_sb[:, k * 512 + w : (k + 1) * 512], mk[:, w:512]
                        )

                mx = pa1.tile([128, 1], F32, tag="mx")
                nc.vector.reduce_max(mx[:], S_sb[:, 0:W], axis=mybir.AxisListType.X)
                negb = pa1.tile([128, 1], F32, tag="negb")
                nc.vector.tensor_scalar_mul(negb[:], mx[:], -1.0 / 32.0)
                P_sb = pa.tile([128, 2048], BF16, tag="P")
                rs = pa1.tile([128, 1], F32, tag="rs")
                nc.scalar.activation(
                    P_sb[:, 0:W],
                    S_sb[:, 0:W],
                    mybir.ActivationFunctionType.Exp,
                    bias=negb[:],
                    scale=1.0 / 32.0,
                    accum_out=rs[:],
                )
                state[l] = (P_sb, rs)

            def emit_pv(l):
                chunks = CHUNKS[l]
                lsl = slice(l * 128, (l + 1) * 128)
                P_sb, rs = state.pop(l)
                oacc = [psO.tile([128, 512], F32, name=f"oacc{h}", tag=f"oacc{h}") for h in range(2)]
                blocks = PVBLK[l]
                for i, q in enumerate(blocks):
                    vj = chunks[q // 4] * 4 + (q % 4)
                    pst = psT.tile([128, 128], BF16, tag="pst")
                    nc.tensor.transpose(
                        pst[:], P_sb[:, q * 128 : (q + 1) * 128], identb[:]
                    )
                    pt = pa1.tile([128, 128], BF16, tag="pt")
                    nc.vector.tensor_copy(pt[:], pst[:])
                    for half in range(2):
                        nc.tensor.matmul(
                            oacc[half][:],
                            pt[:],
                            V[vj][:, half * 512 : (half + 1) * 512],
                            start=(i == 0),
                            stop=(i == len(blocks) - 1),
                        )

                rec = pa1.tile([128, 1], F32, tag="rec")
                nc.vector.reciprocal(rec[:], rs[:])
                for half in range(2):
                    o_sb = pa1.tile([128, 512], F32, tag="o")
                    nc.vector.tensor_scalar_mul(o_sb[:], oacc[half][:], rec[:])
                    nc.sync.dma_start(
                        out_d[lsl, half * 512 : (half + 1) * 512],
                        o_sb[:],
                    )

            # Descending P@V cost order: the pipeline drain after the last
            # score matmuls is then the CHEAPEST block's softmax+P@V
            # (3 blocks) instead of the full 16-block one.
            order = list(range(7, -1, -1))
            for i in range(9):
                if i < 8:
                    emit_scores(order[i])
                if i >= 1:
                    emit_pv(order[i - 1])

    nc.compile()
    _CACHE["nc"] = nc
    return nc


def _rtn22(a):
    """Round fp32 to fp22 (e8m13) with round-to-nearest on host.  The PE
    reads f32r operands truncated to fp22; pre-rounding makes that read
    lossless and replaces truncation bias with unbiased RTN error."""
    u = np.ascontiguousarray(a, dtype=np.float32).view(np.uint32)
    u = (u + np.uint32(0x200)) & np.uint32(0xFFFFFC00)
    return u.view(np.float32)


def _core_inputs(x, Wq, Wk, Wv, c):
    b = c // 2
    my = ABLK if c % 2 == 0 else BBLK
    perm = _perm_rows(my)
    gi = np.concatenate([np.arange(g * 128, (g + 1) * 128) for g in my])
    mask = np.where(perm[None, :] <= gi[:, None] + 1, 0.0, NEG).astype(
        ml_dtypes.bfloat16
    )
    key = ("m2", id(Wq), id(Wk))
    if _CACHE.get("m2key") != key:
        _CACHE["m2"] = _rtn22(
            (Wq.astype(np.float64) @ Wk.T.astype(np.float64)).astype(np.float32)
        )
        _CACHE["m2key"] = key
    return {
        "xt_perm": _rtn22(np.ascontiguousarray(x[b].T[:, perm])),
        "m2": _CACHE["m2"],
        "wv": _rtn22(Wv),
        "maskb": mask,
    }, (b, my)


def kernel(x, Wq, Wk, Wv):
    x = np.ascontiguousarray(np.asarray(x, dtype=np.float32))
    Wq = np.ascontiguousarray(np.asarray(Wq, dtype=np.float32))
    Wk = np.ascontiguousarray(np.asarray(Wk, dtype=np.float32))
    Wv = np.ascontiguousarray(np.asarray(Wv, dtype=np.float32))

    # Recompute M2 unconditionally for this call's weights (the id-based
    # cache in _core_inputs then only dedups the 8 per-core calls below).
    _CACHE["m2"] = _rtn22(
        (Wq.astype(np.float64) @ Wk.T.astype(np.float64)).astype(np.float32)
    )
    _CACHE["m2key"] = ("m2", id(Wq), id(Wk))

    nc = _build()

    in_maps = []
    metas = []
    for c in range(NCORES):
        m, meta = _core_inputs(x, Wq, Wk, Wv, c)
        in_maps.append(m)
        metas.append(meta)

    res = run_bass_kernel_spmd(nc, in_maps, list(range(NCORES)))

    out = np.empty((B, S, DA), dtype=np.float32)
    for c in range(NCORES):
        b, my = metas[c]
        o = res.results[c]["out"]
        for l, g in enumerate(my):
            out[b, g * 128 : (g + 1) * 128] = o[l * 128 : (l + 1) * 128]
    return out


# revision 28
# speedup vs baseline: 1.3170x; 1.0230x over previous
"""Causal attention (single head, d=1024) on 8 trn2 NeuronCores.

Problem: x[4,2048,1024], Wq/Wk/Wv[1024,1024] fp32;
out = softmax(mask(QK^T)/sqrt(1024)) @ V with mask j <= i+1.

Sharding: 2 cores per batch. Causal row work grows ~linearly with row
index, so the two cores split the 16 row-blocks of 128 as
{g : g%4 in {0,3}} vs {g : g%4 in {1,2}} (balanced). Each core receives
x[b]^T with its own rows' columns permuted to the front so that every
core runs the same SPMD program; causality is enforced by a per-core
additive mask tensor (data, not code).

Key algebraic restructure: S = (x Wq)(x Wk)^T = x (Wq Wk^T) x^T.
M2 = Wq Wk^T is batch-independent and computed on the HOST, so the
device never computes K at all: U = x_own @ M2 (one Q-sized projection),
then S = U @ x^T against the resident x^T. This removes the whole
K-projection phase (2048x1024x1024 MACs per core).

Precision: U/S matmuls run as single-pass float32r (PE truncates reads
to fp22 = e8m13, fp32 accumulate; 1 cycle/row for moving dim >= 512).
Host inputs are pre-rounded to fp22 RTN; U is rounded to fp22 RTN on
device via a Veltkamp split so the PE read is lossless. V and P are
bf16 (output budget is lenient). Measured end-to-end relative error
~7e-3 against the fp32 reference (gate 2e-2).

Structure: x^T (8 MB) is DMA'd once into resident SBUF tiles in 512-col
chunks; V then U projections read it in place; attention row-blocks run
last, software-pipelined so softmax of block l overlaps score matmuls
of block l+1, with fully-masked 128-col P blocks skipped in P@V.
"""

import numpy as np
import ml_dtypes

import concourse.bass as bass
import concourse.mybir as mybir
import concourse.tile as tile
from concourse import bacc, masks
from concourse.bass_utils import run_bass_kernel_spmd

B, S, D, DA = 4, 2048, 1024, 1024
NCORES = 8
NBLK = S // 128  # 16 row blocks per batch
F32 = mybir.dt.float32
F32R = mybir.dt.float32r
BF16 = mybir.dt.bfloat16

ABLK = [g for g in range(NBLK) if g % 4 in (0, 3)]
BBLK = [g for g in range(NBLK) if g % 4 in (1, 2)]

NEG = -1e30


def _perm_rows(my):
    oth = [g for g in range(NBLK) if g not in my]
    idx = []
    for g in my + oth:
        idx.extend(range(g * 128, (g + 1) * 128))
    return np.array(idx, dtype=np.int64)


def _chunk_schedule():
    """Per local row-block l: which 512-col chunks of the permuted S row
    must be computed (union over the two roles, so the program is SPMD)."""
    sched = []
    for l in range(8):
        need = [False] * 4
        for my in (ABLK, BBLK):
            perm = _perm_rows(my)  # permuted col -> global row
            jmax = my[l] * 128 + 127 + 1  # max attended global col
            attended = perm <= jmax
            for ch in range(4):
                if attended[ch * 512 : (ch + 1) * 512].any():
                    need[ch] = True
        sched.append([ch for ch in range(4) if need[ch]])
    return sched


CHUNKS = _chunk_schedule()


def _pv_schedule():
    """Per local row-block l: which packed 128-col blocks of P (positions
    within the packed CHUNKS[l] layout) have any unmasked column for either
    role (union -> SPMD).  Blocks that are fully masked produce P == 0 and
    can be skipped in the P@V accumulation."""
    out = []
    for l in range(8):
        chunks = CHUNKS[l]
        needset = set()
        for my in (ABLK, BBLK):
            perm = _perm_rows(my)
            jmax = my[l] * 128 + 127 + 1
            attended = perm <= jmax
            for k, ch in enumerate(chunks):
                for q in range(4):
                    blk = ch * 4 + q
                    if attended[blk * 128 : (blk + 1) * 128].any():
                        needset.add(k * 4 + q)
        out.append(sorted(needset))
    return out


PVBLK = _pv_schedule()


def _score_widths():
    """Per (l, chunk k): moving width for the score matmul.  Within a chunk
    the needed 128-col blocks always form a prefix (union of two per-role
    prefixes), so the matmul can stop early; f32r needs moving >= 256 to
    stay at 1 cycle/row.  The trimmed tail is filled from the mask
    (-1e30 -> exp 0), keeping the packed P layout unchanged."""
    out = []
    for l in range(8):
        nch = len(CHUNKS[l])
        widths = []
        for k in range(nch):
            n = sum(1 for q in PVBLK[l] if k * 4 <= q < (k + 1) * 4)
            widths.append(min(512, max(256, 128 * n)))
        out.append(widths)
    return out


SWID = _score_widths()

_CACHE = {}


def _build():
    if "nc" in _CACHE:
        return _CACHE["nc"]

    nc = bacc.Bacc()
    # Inputs declared float32r (bit-identical to f32) so non-casting DMA
    # queues can load them.
    xt_d = nc.dram_tensor("xt_perm", [D, S], F32R, kind="ExternalInput")
    m2_d = nc.dram_tensor("m2", [D, D], F32R, kind="ExternalInput")
    wv_d = nc.dram_tensor("wv", [D, DA], F32R, kind="ExternalInput")
    mask_d = nc.dram_tensor("maskb", [1024, S], BF16, kind="ExternalInput")
    out_d = nc.dram_tensor("out", [1024, DA], F32, kind="ExternalOutput")

    from contextlib import ExitStack

    with tile.TileContext(nc) as tc, ExitStack() as stack:
        cpool = stack.enter_context(tc.tile_pool(name="const", bufs=1))
        identb = cpool.tile([128, 128], BF16, tag="identb")
        masks.make_identity(nc, identb[:])

        # long-lived residents (live until the end of attention)
        vpool = stack.enter_context(tc.tile_pool(name="vres", bufs=1))
        V = [vpool.tile([128, DA], BF16, name=f"v{j}", tag=f"v{j}") for j in range(16)]
        upool = stack.enter_context(tc.tile_pool(name="utres", bufs=1))
        UT = [upool.tile([128, 1024], F32R, name=f"ut{a}", tag=f"ut{a}") for a in range(8)]
        xpool = stack.enter_context(tc.tile_pool(name="xtres", bufs=1))
        XT = [xpool.tile([128, S], F32R, name=f"xt{d}", tag=f"xt{d}") for d in range(8)]

        # x^T loaded once, in 512-col chunks so consumers start early
        for jc in range(4):
            for d in range(8):
                nc.sync.dma_start(
                    XT[d][:, jc * 512 : (jc + 1) * 512],
                    xt_d[d * 128 : (d + 1) * 128, jc * 512 : (jc + 1) * 512],
                )

        # ---- Projections: V (all rows) then U = x_own @ M2 ---------------
        with (
            tc.tile_pool(name="wproj", bufs=1) as pw,
            tc.tile_pool(name="m2w", bufs=1) as pm,
            tc.tile_pool(name="vtmp", bufs=2) as ptmp,
            tc.tile_pool(name="psproj", bufs=4, space="PSUM") as pps,
            tc.tile_pool(name="psdmy", bufs=1, space="PSUM") as pdm,
        ):
            wv = [pw.tile([128, DA], F32R, name=f"wv{d}", tag=f"wv{d}") for d in range(8)]
            m2 = [pm.tile([128, D], F32R, name=f"m2{d}", tag=f"m2{d}") for d in range(8)]
            for d in range(8):
                nc.gpsimd.dma_start(wv[d][:], wv_d[d * 128 : (d + 1) * 128, :])
                nc.scalar.dma_start(m2[d][:], m2_d[d * 128 : (d + 1) * 128, :])

            # HAM warm-up: the first real matmul can't start until ~14us of
            # input DMA lands; these dummy matmuls run during that window
            # and keep the PE activity monitor at full clock (idle >3.4us
            # re-throttles the PE to 1.2 GHz), so real matmuls start warm.
            dmy = pdm.tile([128, 128], F32, tag="dmy")
            for _ in range(100):
                nc.tensor.matmul(dmy[:], identb[:], identb[:], start=True, stop=True)

            def round13(dst, ps):
                # Veltkamp split: round PSUM fp32 to 14-bit significand
                # (e8m13) round-to-nearest, so the PE's f32r read of dst is
                # lossless.
                c = ptmp.tile([128, 512], F32, tag="vc")
                dd = ptmp.tile([128, 512], F32, tag="vd")
                nc.vector.tensor_scalar_mul(c[:], ps[:], 1025.0)
                nc.vector.tensor_sub(dd[:], c[:], ps[:])
                nc.vector.tensor_sub(dst, c[:], dd[:])

            # V (all rows)
            for jc in range(4):
                for q in range(4):
                    vj = jc * 4 + q
                    jsl = slice(vj * 128, (vj + 1) * 128)
                    for half in range(2):
                        ps = pps.tile([128, 512], F32, tag="psp")
                        for d in range(8):
                            nc.tensor.matmul(
                                ps[:],
                                XT[d][:, jsl],
                                wv[d][:, half * 512 : (half + 1) * 512],
                                start=(d == 0),
                                stop=(d == 7),
                            )
                        nc.vector.tensor_copy(
                            V[vj][:, half * 512 : (half + 1) * 512], ps[:]
                        )

            # U^T = M2^T x_own^T (own rows = first two chunks).  jc=1 first:
            # attention processes row-blocks 7..0, and block 7 reads the
            # jc=1 half of UT, so its scores never wait on the U tail.
            for jc in (1, 0):
                csl = slice(jc * 512, (jc + 1) * 512)
                for a in range(8):
                    ps = pps.tile([128, 512], F32, tag="psp")
                    for d in range(8):
                        nc.tensor.matmul(
                            ps[:],
                            m2[d][:, a * 128 : (a + 1) * 128],
                            XT[d][:, csl],
                            start=(d == 0),
                            stop=(d == 7),
                        )
                    round13(UT[a][:, csl], ps)

        # ---- Attention per local row-block, software-pipelined -----------
        with (
            tc.tile_pool(name="attn", bufs=2) as pa,
            tc.tile_pool(name="attn1", bufs=2) as pa1,
            tc.tile_pool(name="psS", bufs=2, space="PSUM") as psS,
            tc.tile_pool(name="psT", bufs=2, space="PSUM") as psT,
            tc.tile_pool(name="psO", bufs=2, space="PSUM") as psO,
        ):
            # stage state carried from score/softmax stage to PV stage
            state = {}

            def emit_scores(l):
                chunks = CHUNKS[l]
                W = len(chunks) * 512
                lsl = slice(l * 128, (l + 1) * 128)
                S_sb = pa.tile([128, 2048], F32, tag="S")
                for k, ch in enumerate(chunks):
                    w = SWID[l][k]
                    ps = psS.tile([128, 512], F32, tag="ps")
                    csl = slice(ch * 512, ch * 512 + w)
                    for ac in range(8):
                        nc.tensor.matmul(
                            ps[:, 0:w],
                            UT[ac][:, lsl],
                            XT[ac][:, csl],
                            start=(ac == 0),
                            stop=(ac == 7),
                        )
                    mk = pa1.tile([128, 512], BF16, tag="mk")
                    nc.gpsimd.dma_start(mk[:], mask_d[lsl, ch * 512 : (ch + 1) * 512])
                    nc.vector.tensor_add(
                        S_sb[:, k * 512 : k * 512 + w], ps[:, 0:w], mk[:, 0:w]
                    )
                    if w < 512:
                        # fully-masked tail: pure mask (-1e30), exp -> 0
                        nc.vector.tensor_copy(
                            S_sb[:, k * 512 + w : (k + 1) * 512], mk[:, w:512]
                        )

                mx = pa1.tile([128, 1], F32, tag="mx")
                nc.vector.reduce_max(mx[:], S_sb[:, 0:W], axis=mybir.AxisListType.X)
                negb = pa1.tile([128, 1], F32, tag="negb")
                nc.vector.tensor_scalar_mul(negb[:], mx[:], -1.0 / 32.0)
                P_sb = pa.tile([128, 2048], BF16, tag="P")
                rs = pa1.tile([128, 1], F32, tag="rs")
                nc.scalar.activation(
                    P_sb[:, 0:W],
                    S_sb[:, 0:W],
                    mybir.ActivationFunctionType.Exp,
                    bias=negb[:],
                    scale=1.0 / 32.0,
                    accum_out=rs[:],
                )
                state[l] = (P_sb, rs)

            def emit_pv(l):
                chunks = CHUNKS[l]
                lsl = slice(l * 128, (l + 1) * 128)
                P_sb, rs = state.pop(l)
                oacc = [psO.tile([128, 512], F32, name=f"oacc{h}", tag=f"oacc{h}") for h in range(2)]
                blocks = PVBLK[l]
                for i, q in enumerate(blocks):
                    vj = chunks[q // 4] * 4 + (q % 4)
                    pst = psT.tile([128, 128], BF16, tag="pst")
                    nc.tensor.transpose(
                        pst[:], P_sb[:, q * 128 : (q + 1) * 128], identb[:]
                    )
                    pt = pa1.tile([128, 128], BF16, tag="pt")
                    nc.vector.tensor_copy(pt[:], pst[:])
                    for half in range(2):
                        nc.tensor.matmul(
                            oacc[half][:],
                            pt[:],
                            V[vj][:, half * 512 : (half + 1) * 512],
                            start=(i == 0),
                            stop=(i == len(blocks) - 1),
                        )

                rec = pa1.tile([128, 1], F32, tag="rec")
                nc.vector.reciprocal(rec[:], rs[:])
                for half in range(2):
                    o_sb = pa1.tile([128, 512], F32, tag="o")
                    nc.vector.tensor_scalar_mul(o_sb[:], oacc[half][:], rec[:])
                    nc.sync.dma_start(
                        out_d[lsl, half * 512 : (half + 1) * 512],
                        o_sb[:],
                    )

            # Descending P@V cost order: the pipeline drain after the last
            # score matmuls is then the CHEAPEST block's softmax+P@V
            # (3 blocks) instead of the full 16-block one.
            order = list(range(7, -1, -1))
            for i in range(9):
                if i < 8:
                    emit_scores(order[i])
                if i >= 1:
                    emit_pv(order[i - 1])

    nc.compile()
    _CACHE["nc"] = nc
    return nc


def _rtn22(a):
    """Round fp32 to fp22 (e8m13) with round-to-nearest on host.  The PE
    reads f32r operands truncated to fp22; pre-rounding makes that read
    lossless and replaces truncation bias with unbiased RTN error."""
    u = np.ascontiguousarray(a, dtype=np.float32).view(np.uint32)
    u = (u + np.uint32(0x200)) & np.uint32(0xFFFFFC00)
    return u.view(np.float32)


def _core_inputs(x, Wq, Wk, Wv, c):
    b = c // 2
    my = ABLK if c % 2 == 0 else BBLK
    perm = _perm_rows(my)
    gi = np.concatenate([np.arange(g * 128, (g + 1) * 128) for g in my])
    mask = np.where(perm[None, :] <= gi[:, None] + 1, 0.0, NEG).astype(
        ml_dtypes.bfloat16
    )
    key = ("m2", id(Wq), id(Wk))
    if _CACHE.get("m2key") != key:
        _CACHE["m2"] = _rtn22(
            (Wq.astype(np.float64) @ Wk.T.astype(np.float64)).astype(np.float32)
        )
        _CACHE["m2key"] = key
    return {
        "xt_perm": _rtn22(np.ascontiguousarray(x[b].T[:, perm])),
        "m2": _CACHE["m2"],
        "wv": _rtn22(Wv),
        "maskb": mask,
    }, (b, my)


def kernel(x, Wq, Wk, Wv):
    x = np.ascontiguousarray(np.asarray(x, dtype=np.float32))
    Wq = np.ascontiguousarray(np.asarray(Wq, dtype=np.float32))
    Wk = np.ascontiguousarray(np.asarray(Wk, dtype=np.float32))
    Wv = np.ascontiguousarray(np.asarray(Wv, dtype=np.float32))

    # Recompute M2 unconditionally for this call's weights (the id-based
    # cache in _core_inputs then only dedups the 8 per-core calls below).
    _CACHE["m2"] = _rtn22(
        (Wq.astype(np.float64) @ Wk.T.astype(np.float64)).astype(np.float32)
    )
    _CACHE["m2key"] = ("m2", id(Wq), id(Wk))

    nc = _build()

    in_maps = []
    metas = []
    for c in range(NCORES):
        m, meta = _core_inputs(x, Wq, Wk, Wv, c)
        in_maps.append(m)
        metas.append(meta)

    res = run_bass_kernel_spmd(nc, in_maps, list(range(NCORES)))

    out = np.empty((B, S, DA), dtype=np.float32)
    for c in range(NCORES):
        b, my = metas[c]
        o = res.results[c]["out"]
        for l, g in enumerate(my):
            out[b, g * 128 : (g + 1) * 128] = o[l * 128 : (l + 1) * 128]
    return out
